# revision 1
# baseline (speedup 1.0000x reference)
"""Multi-head causal attention (B=2, T=2048, D=2048, H=16) on 8 trn2 NeuronCores.

Sharding: tensor-parallel over heads (2 heads/core). x^T is replicated, W_qkv
column-sliced and W_out row-sliced per core; each core computes a full-shape
partial of the output projection and the host sums the 8 partials (+ b_out).

All big matmuls run as float32r (fp32 storage, reduced-precision PE multiply
at full rate). Causal structure: only lower-triangular score tiles are
computed. Softmax runs without max-subtraction (scores provably < ~25, safe
in fp32) and without normalizing P: scores are computed transposed [k, q],
exponentiated, causally zeroed on diagonal blocks (GPSIMD affine_select),
and fed straight to P@V; row-sums accumulate alongside via a ones-column
matmul and the normalization happens once at the O^T eviction.
"""

import math
import os

import numpy as np

import concourse.bass as bass
import concourse.mybir as mybir
import concourse.tile as tile
from concourse import bacc
from concourse.bass_utils import run_bass_kernel_spmd
from concourse.masks import make_identity

B, T, D_IN, D_MODEL, H = 2, 2048, 2048, 2048, 16
DH = 128
NCORES = 8
HPC = H // NCORES  # heads per core
BT = B * T
SCALE = 1.0 / math.sqrt(DH)

F32 = mybir.dt.float32
F32R = mybir.dt.float32r
AF = mybir.ActivationFunctionType
ALU = mybir.AluOpType

TOKT = 512             # stage-1 token tile
NTT = T // TOKT        # token tiles per batch (4)
NDCH = D_IN // 128     # d_in contraction chunks (16)
NQ = T // 128          # 128-token chunks per batch (16)
NJ = T // 512          # q 512-tiles per batch (4)
NFT = D_MODEL // 512   # output feature tiles (4)


def build_nc(debug=False, reps=1):
    nc = bacc.Bacc("TRN2", target_bir_lowering=False, debug=False,
                   num_devices=NCORES)

    xT = nc.dram_tensor("xT", [D_IN, BT], F32R, kind="ExternalInput")
    wq = nc.dram_tensor("wq", [D_IN, HPC * DH], F32R, kind="ExternalInput")
    wk = nc.dram_tensor("wk", [D_IN, HPC * DH], F32R, kind="ExternalInput")
    wv = nc.dram_tensor("wv", [D_IN, HPC * DH], F32R, kind="ExternalInput")
    bq = nc.dram_tensor("bq", [HPC * DH], F32, kind="ExternalInput")
    bk = nc.dram_tensor("bk", [HPC * DH], F32, kind="ExternalInput")
    bv = nc.dram_tensor("bv", [HPC * DH], F32, kind="ExternalInput")
    wo = nc.dram_tensor("wo", [HPC * DH, D_MODEL], F32R, kind="ExternalInput")
    cosT = nc.dram_tensor("cosT", [DH, T], F32, kind="ExternalInput")
    sinTs = nc.dram_tensor("sinTs", [DH, T], F32, kind="ExternalInput")
    y = nc.dram_tensor("y", [BT, D_MODEL], F32, kind="ExternalOutput")

    dbg = {}
    if debug:
        dbg["qT"] = nc.dram_tensor("dbg_qT", [HPC, B, DH, T], F32, kind="ExternalOutput")
        dbg["kT"] = nc.dram_tensor("dbg_kT", [HPC, B, DH, T], F32, kind="ExternalOutput")
        dbg["v"] = nc.dram_tensor("dbg_v", [B, T, HPC * DH], F32, kind="ExternalOutput")
        dbg["ot"] = nc.dram_tensor("dbg_ot", [B, HPC, DH, T], F32, kind="ExternalOutput")

    with tile.TileContext(nc) as tc:
        with (
            tc.tile_pool(name="persist", bufs=1) as pp,
            tc.tile_pool(name="weights", bufs=1) as wp,
            tc.tile_pool(name="qkv", bufs=1) as qp,
        ):
            # ---- per-core weights, quarter 0 first (unblocks first matmuls)
            wq_sb = wp.tile([128, NDCH, HPC * DH], F32R, name="wq_sb")
            wk_sb = wp.tile([128, NDCH, HPC * DH], F32R, name="wk_sb")
            wv_sb = wp.tile([128, NDCH, HPC * DH], F32R, name="wv_sb")

            def _w_quarter(hf):
                for t_, d_ in ((wq_sb, wq), (wk_sb, wk), (wv_sb, wv)):
                    nc.sync.dma_start(
                        t_[:, hf * (NDCH // 4):(hf + 1) * (NDCH // 4), :],
                        d_.ap()[hf * (D_IN // 4):(hf + 1) * (D_IN // 4), :]
                        .rearrange("(c p) f -> p c f", p=128))

            _w_quarter(0)

            # prefetch the very first x^T quarter so tau-0 matmuls start early
            # (skipped in the repeated timing build: a tile allocated outside
            # the loop but read inside would pin its pool slot across reps)
            xs_cm = tc.tile_pool(name="xs", bufs=3)
            xs = xs_cm.__enter__()
            xt00 = None
            if reps == 1:
                xt00 = xs.tile([128, 4, TOKT], F32R, name="xt")
                nc.sync.dma_start(
                    xt00[:],
                    xT.ap()[0:512, 0:TOKT].rearrange("(c p) t -> p c t", p=128))

            # ---- constants (needed ~15us in, after the first accumulations)
            cosT_sb = pp.tile([DH, T], F32, name="cosT_sb")
            sinTs_sb = pp.tile([DH, T], F32, name="sinTs_sb")
            nc.sync.dma_start(cosT_sb[:], cosT.ap())
            nc.sync.dma_start(sinTs_sb[:], sinTs.ap())
            ones1 = pp.tile([1, 128], F32, name="ones1")
            nc.gpsimd.memset(ones1[:], 1.0)
            onescol = pp.tile([128, 1], F32, name="onescol")
            nc.gpsimd.memset(onescol[:], 1.0)
            onescol_r = pp.tile([128, 1], F32R, name="onescol_r")
            nc.scalar.copy(onescol_r[:], onescol[:])
            ident = pp.tile([128, 128], F32, name="ident")
            make_identity(nc, ident[:])
            bqt = pp.tile([128, HPC], F32, name="bqt")
            bkt = pp.tile([128, HPC], F32, name="bkt")
            bvt = pp.tile([128, HPC], F32, name="bvt")
            nc.sync.dma_start(bqt[:], bq.ap().rearrange("(h d) -> d h", d=DH))
            nc.sync.dma_start(bkt[:], bk.ap().rearrange("(h d) -> d h", d=DH))
            nc.sync.dma_start(bvt[:], bv.ap().rearrange("(h d) -> d h", d=DH))

            for hf in range(1, 4):
                _w_quarter(hf)

            # ---- per-batch Q^T/K^T/V and O^T buffers -----------------------
            qT_sb = [qp.tile([DH, T], F32R, name=f"qT{h}") for h in range(HPC)]
            kT_sb = [qp.tile([DH, T], F32R, name=f"kT{h}") for h in range(HPC)]
            v_sb = qp.tile([128, NQ, HPC * DH], F32R, name="v_sb")
            ot_sb = [[pp.tile([DH, T], F32R, name=f"ot{b}_{h}") for h in range(HPC)]
                     for b in range(B)]

            import contextlib
            rep_ctx = (tc.For_i(0, reps, 1, hint_engines=(
                mybir.EngineType.PE, mybir.EngineType.Activation,
                mybir.EngineType.DVE, mybir.EngineType.Pool,
                mybir.EngineType.SP))
                if reps > 1 else contextlib.nullcontext())
            with rep_ctx:
                _emit_body(nc, tc, xT, wq_sb, wk_sb, wv_sb, bqt, bkt, bvt,
                           cosT_sb, sinTs_sb, qT_sb, kT_sb, v_sb, ot_sb,
                           wo, y, ones1, onescol_r, ident, dbg, xs, xt00)
            xs_cm.__exit__(None, None, None)
    nc.compile()
    return nc


def _emit_body(nc, tc, xT, wq_sb, wk_sb, wv_sb, bqt, bkt, bvt, cosT_sb,
               sinTs_sb, qT_sb, kT_sb, v_sb, ot_sb, wo, y, ones1,
               onescol_r, ident, dbg, xs, xt00):
    wop_cm = tc.tile_pool(name="wo_p", bufs=1)
    wop = wop_cm.__enter__()
    wo_sb = None
    ypools = {}

    def emit_y(b):
        if not ypools:
            ypools["yp_cm"] = tc.tile_pool(name="y_p", bufs=4)
            ypools["yp"] = ypools["yp_cm"].__enter__()
            ypools["yps_cm"] = tc.tile_pool(name="y_ps", bufs=4, space="PSUM")
            ypools["yps"] = ypools["yps_cm"].__enter__()
        yp, yps = ypools["yp"], ypools["yps"]
        for tt in range(NQ):
            for ft in range(NFT):
                ps = yps.tile([128, 512], F32, name="y_acc")
                for h in range(HPC):
                    nc.tensor.matmul(
                        ps[:], ot_sb[b][h][:, tt * 128:(tt + 1) * 128],
                        wo_sb[:, h, ft * 512:(ft + 1) * 512],
                        start=(h == 0), stop=(h == HPC - 1))
                yt = yp.tile([128, 512], F32, name="y_t")
                nc.scalar.copy(yt[:], ps[:])
                nc.sync.dma_start(
                    y.ap()[b * T + tt * 128:b * T + (tt + 1) * 128,
                           ft * 512:(ft + 1) * 512],
                    yt[:])

    for b in range(B):
        _stage1(nc, tc, b, xT, wq_sb, wk_sb, wv_sb, bqt, bkt, bvt,
                cosT_sb, sinTs_sb, qT_sb, kT_sb, v_sb, ident, xs,
                xt00 if b == 0 else None)
        if dbg:
            for h in range(HPC):
                nc.sync.dma_start(dbg["qT"].ap()[h, b], qT_sb[h][:].bitcast(F32))
                nc.sync.dma_start(dbg["kT"].ap()[h, b], kT_sb[h][:].bitcast(F32))
            nc.sync.dma_start(
                dbg["v"].ap()[b].rearrange("(c p) f -> p c f", p=128),
                v_sb[:].bitcast(F32))
        _stage2(nc, tc, b, qT_sb, kT_sb, v_sb, ones1, onescol_r,
                ot_sb, dbg)
        if b == 0:
            # prefetch W_out during the second batch's compute
            wo_sb = wop.tile([128, HPC, D_MODEL], F32R, name="wo_sb")
            nc.sync.dma_start(wo_sb[:],
                               wo.ap().rearrange("(h p) f -> p h f", p=128))

    if dbg:
        for bb in range(B):
            for h in range(HPC):
                nc.sync.dma_start(dbg["ot"].ap()[bb, h],
                                  ot_sb[bb][h][:].bitcast(F32))
    emit_y(0)
    emit_y(1)
    ypools["yps_cm"].__exit__(None, None, None)
    ypools["yp_cm"].__exit__(None, None, None)
    wop_cm.__exit__(None, None, None)


def _stage1(nc, tc, b, xT, wq_sb, wk_sb, wv_sb, bqt, bkt, bvt,
            cosT_sb, sinTs_sb, qT_sb, kT_sb, v_sb, ident, xs, xt00):
    """QKV projection + RoPE for batch b: fills qT_sb/kT_sb/v_sb.

    Loop nest is d_in-chunk-outer so each x^T quarter-tile is touched once.
    q/k/v are all computed transposed ([feat, tok], N=512, weight loads fully
    hidden); V is then rotated back to natural [tok, feat] layout with PE
    transposes so it can serve as the stationary operand of P@V.
    """
    with (
        tc.tile_pool(name="st", bufs=2) as st,
        tc.tile_pool(name="vt", bufs=2) as vtp,
        tc.tile_pool(name="ps_qk", bufs=4, space="PSUM") as psqk,
        tc.tile_pool(name="ps_v", bufs=2, space="PSUM") as psv,
        tc.tile_pool(name="ps_tr", bufs=2, space="PSUM") as pstr,
    ):
        for tau in range(NTT):
            pos = tau * TOKT
            gtok = b * T + pos
            accs = [psqk.tile([128, TOKT], F32, name="qk_acc") for _ in range(4)]
            accvT = [psv.tile([128, TOKT], F32, name="vT_acc") for _ in range(2)]
            for quarter in range(4):
                if tau == 0 and quarter == 0 and xt00 is not None:
                    xt = xt00
                else:
                    xt = xs.tile([128, 4, TOKT], F32R, name="xt")
                    nc.sync.dma_start(
                        xt[:],
                        xT.ap()[quarter * 512:(quarter + 1) * 512,
                                gtok:gtok + TOKT]
                        .rearrange("(c p) t -> p c t", p=128))
                for cl in range(4):
                    c = quarter * 4 + cl
                    for fi, (wsb, hh) in enumerate(
                            ((wq_sb, 0), (wq_sb, 1), (wk_sb, 0), (wk_sb, 1))):
                        nc.tensor.matmul(
                            accs[fi][:], wsb[:, c, hh * DH:(hh + 1) * DH],
                            xt[:, cl, :],
                            start=(c == 0), stop=(c == NDCH - 1))
                    for hh in range(HPC):
                        nc.tensor.matmul(
                            accvT[hh][:], wv_sb[:, c, hh * DH:(hh + 1) * DH],
                            xt[:, cl, :],
                            start=(c == 0), stop=(c == NDCH - 1))
            # q/k evictions with bias (split ACT/DVE), then RoPE on DVE
            for fi, (bias, dest, hh) in enumerate(
                    ((bqt, qT_sb, 0), (bqt, qT_sb, 1),
                     (bkt, kT_sb, 0), (bkt, kT_sb, 1))):
                stg = st.tile([128, TOKT], F32, name="stg")
                if fi < 2:
                    nc.scalar.activation(stg[:], accs[fi][:], AF.Identity,
                                         bias=bias[:, hh:hh + 1], scale=1.0)
                else:
                    nc.vector.tensor_scalar_add(stg[:], accs[fi][:],
                                                bias[:, hh:hh + 1])
                rot = st.tile([128, TOKT], F32, name="rot")
                nc.scalar.copy(rot[0:64, :], stg[64:128, :])
                nc.scalar.copy(rot[64:128, :], stg[0:64, :])
                nc.vector.tensor_tensor(
                    stg[:], stg[:], cosT_sb[:, pos:pos + TOKT], ALU.mult)
                nc.vector.tensor_tensor(
                    rot[:], rot[:], sinTs_sb[:, pos:pos + TOKT], ALU.mult)
                nc.vector.tensor_tensor(
                    dest[hh][:, pos:pos + TOKT], stg[:], rot[:], ALU.add)
            # V: evict V^T with bias, then PE-transpose back to natural layout
            # (transpose runs in plain fp32 — the f32r LDW path is broken in
            # walrus codegen; rounding to f32r happens in the PSUM eviction)
            for hh in range(HPC):
                vt = vtp.tile([128, TOKT], F32, name="vt")
                nc.scalar.activation(vt[:], accvT[hh][:], AF.Identity,
                                     bias=bvt[:, hh:hh + 1], scale=1.0)
                for ts in range(4):
                    tr = pstr.tile([128, 128], F32, name="tr")
                    nc.tensor.transpose(tr[:], vt[:, ts * 128:(ts + 1) * 128],
                                        ident[:])
                    nc.scalar.copy(
                        v_sb[:, (pos // 128) + ts, hh * DH:(hh + 1) * DH], tr[:])


def _stage2(nc, tc, b, qT_sb, kT_sb, v_sb, ones1, onescol_r, ot_sb, dbg):
    """Causal attention for batch b, both heads interleaved: fills ot_sb[b].

    Single pass per tile: S^T -> exp -> causal zero (diag blocks, GPSIMD) ->
    P@V accumulation + ones-matmul row-sum accumulation; O^T normalized by
    1/rowsum (PE-broadcast along partitions) during eviction. The two heads
    alternate per (j, kk) step so one head's exp latency hides under the
    other head's matmuls.
    """
    with (
        tc.tile_pool(name="spsB", bufs=3, space="PSUM") as spsB,
        tc.tile_pool(name="rps", bufs=2, space="PSUM") as rps,
        tc.tile_pool(name="ops", bufs=2, space="PSUM") as ops,
        tc.tile_pool(name="scr", bufs=2) as scr,
        tc.tile_pool(name="pt_p", bufs=5) as ptp,
    ):
        for j in range(NJ):
            nkk = 4 * j + 4
            rp = [rps.tile([1, 512], F32, name="r_ps") for _ in range(HPC)]
            op = [ops.tile([128, 512], F32, name="o_ps") for _ in range(HPC)]
            for kk in range(nkk):
                for h in range(HPC):
                    qT, kT = qT_sb[h], kT_sb[h]
                    sp = spsB.tile([128, 512], F32, name="st_ps")
                    nc.tensor.matmul(sp[:], kT[:, kk * 128:(kk + 1) * 128],
                                     qT[:, j * 512:(j + 1) * 512],
                                     start=True, stop=True)
                    pt = ptp.tile([128, 512], F32R, name="pt")
                    nc.scalar.activation(pt[:], sp[:], AF.Exp, bias=0.0,
                                         scale=SCALE)
                    if kk // 4 == j:
                        # zero entries with q < k: keep where f - p - off >= 0
                        nc.gpsimd.affine_select(
                            out=pt[:], in_=pt[:], compare_op=ALU.is_ge,
                            fill=0.0, base=-(kk % 4) * 128, pattern=[[1, 512]],
                            channel_multiplier=-1)
                    nc.tensor.matmul(op[h][:],
                                     v_sb[:, kk, h * DH:(h + 1) * DH],
                                     pt[:], start=(kk == 0),
                                     stop=(kk == nkk - 1))
                    nc.tensor.matmul(rp[h][:], onescol_r[:], pt[:],
                                     start=(kk == 0), stop=(kk == nkk - 1))
            # rowsum -> reciprocal -> broadcast across partitions -> evict
            for h in range(HPC):
                rrow_inv = scr.tile([1, 512], F32, name="rrow_inv")
                nc.vector.reciprocal(rrow_inv[:], rp[h][:])
                rb_ps = spsB.tile([128, 512], F32, name="st_ps", tag="st_ps")
                nc.tensor.matmul(rb_ps[:], ones1[:], rrow_inv[:],
                                 start=True, stop=True)
                rb = scr.tile([128, 512], F32, name="rb")
                nc.scalar.copy(rb[:], rb_ps[:])
                nc.vector.tensor_tensor(ot_sb[b][h][:, j * 512:(j + 1) * 512],
                                        op[h][:], rb[:], ALU.mult)


_CACHE = {}


def _get_nc():
    if "nc" not in _CACHE:
        _CACHE["nc"] = build_nc(debug=bool(int(os.environ.get("KERNEL_DEBUG", "0"))))
    return _CACHE["nc"]


def _host_prep(x, W_qkv, b_qkv, W_out, mask):
    xT = np.ascontiguousarray(x.reshape(BT, D_IN).T)
    Wr = W_qkv.reshape(D_IN, H, 3, DH)
    br = b_qkv.reshape(H, 3, DH)
    # RoPE tables, transposed, sign-folded (rows 0:64 of sinTs negated)
    inv_freq = (1.0 / (10000.0 ** (np.arange(0, DH, 2, dtype=np.float32) / DH))).astype(np.float32)
    tpos = np.arange(T, dtype=np.float32)
    freqs = tpos[:, None] * inv_freq[None, :]              # (T, 64)
    emb = np.concatenate([freqs, freqs], axis=-1)          # (T, 128)
    cosT = np.ascontiguousarray(np.cos(emb).astype(np.float32).T)
    sinT = np.sin(emb).astype(np.float32).T
    sinTs = sinT.copy()
    sinTs[0:64] = -sinTs[0:64]
    sinTs = np.ascontiguousarray(sinTs)

    in_maps = []
    for i in range(NCORES):
        hs = [HPC * i + k for k in range(HPC)]
        in_maps.append({
            "xT": xT,
            "wq": np.ascontiguousarray(Wr[:, hs, 0, :].reshape(D_IN, HPC * DH)),
            "wk": np.ascontiguousarray(Wr[:, hs, 1, :].reshape(D_IN, HPC * DH)),
            "wv": np.ascontiguousarray(Wr[:, hs, 2, :].reshape(D_IN, HPC * DH)),
            "bq": np.ascontiguousarray(br[hs, 0, :].reshape(HPC * DH)),
            "bk": np.ascontiguousarray(br[hs, 1, :].reshape(HPC * DH)),
            "bv": np.ascontiguousarray(br[hs, 2, :].reshape(HPC * DH)),
            "wo": np.ascontiguousarray(W_out[hs[0] * DH:(hs[-1] + 1) * DH, :]),
            "cosT": cosT,
            "sinTs": sinTs,
        })
    return in_maps


def kernel(x, W_qkv, b_qkv, W_out, b_out, mask):
    x = np.asarray(x, dtype=np.float32)
    in_maps = _host_prep(np.asarray(x), np.asarray(W_qkv), np.asarray(b_qkv),
                         np.asarray(W_out), np.asarray(mask))
    nc = _get_nc()
    res = run_bass_kernel_spmd(nc, in_maps, core_ids=list(range(NCORES)))
    out = res.results[0]["y"].copy()
    for i in range(1, NCORES):
        out += res.results[i]["y"]
    out += np.asarray(b_out, dtype=np.float32)[None, :]
    return out.reshape(B, T, D_MODEL).astype(np.float32)

